# revision 3
# baseline (speedup 1.0000x reference)
"""Batched 3-layer GCN on 8 TRN2 NeuronCores — one graph per core.

Same cell-grid architecture as v1 (80x80 src x dest blocks of 32 slots,
grid transpose through DRAM, overflow via per-edge dma_gather), with:
  - degree pass removed: dinv = (deg>0)?deg^-1/2:0 is pure index data
    (bincount of edge_index rows) — computed host-side and shipped as a
    [128, 80] node-major constant
  - z-phase restructured: per-block matmul with stationary h (f-major,
    bf16, 65th ones-row folds the bias in) and moving W.T produces
    node-major z directly — no PE transposes; dinv scaling fused into
    the PSUM->fp16 DVE drain (8 blocks per PSUM bank)
  - scatter drain handoff transposed via a plain matmul against an
    identity moving operand (~90ns) instead of PE transpose-mode (~275ns)
  - h carried in bf16 (stationary FWL eligible), gather one-hots shipped
    fp8e4 (exact for 0/1; halves their HBM traffic, 4x FWL weight load)
Messages stay fp16; accumulation fp32 in PSUM.
"""
from dataclasses import dataclass

import numpy as np

import concourse.bacc as bacc
import concourse.mybir as mybir
import concourse.tile as tile
from concourse.bass import broadcast_tensor_aps
from concourse.bass_utils import run_bass_kernel_spmd
from concourse.library_config import mlp

OH_FP8 = True      # scatter one-hots in fp8e4 (exact 0/1, FWL 4x)
OH_SHIP = 2        # 0: build all on DVE; 1: ship even dp pairs; 2: ship all

B, NV, E, F = 8, 10000, 160000, 64
N = 10240
NBLK = 80
CELL = 32
SROW = NBLK * CELL          # 2560 slots per src-block row (= per dest block)
SCH = SROW // 128           # 20 gather-matmul chunks per src block
EPADG = NBLK * SROW         # 204800 grid slots
CORES = list(range(8))


@dataclass(frozen=True)
class Cfg:
    OV_CB: int              # overflow chunks (of 128 slots) per dest block
    OVG: int = 4            # dest blocks per overflow gather call
    layers: int = 3

    @property
    def epadov(self):
        return NBLK * self.OV_CB * 128


def make_cfg(edge_index):
    mx = 0
    for g in range(edge_index.shape[0]):
        row = np.asarray(edge_index[g, :, 0], np.int64)
        col = np.asarray(edge_index[g, :, 1], np.int64)
        cell = (col >> 7) * NBLK + (row >> 7)
        ccnt = np.bincount(cell, minlength=NBLK * NBLK)
        ovper = np.maximum(ccnt - CELL, 0).reshape(NBLK, NBLK).sum(axis=0)
        mx = max(mx, int(ovper.max()))
    return Cfg(OV_CB=max(1, -(-mx // 128)))


def _build(cfg: Cfg, trips: int = 1):
    OV_CB = cfg.OV_CB
    EPADOV = cfg.epadov
    OVG = cfg.OVG

    nc = bacc.Bacc("TRN2", debug=False)
    h_hbm = nc.dram_tensor("h0_ext", [65, N], mybir.dt.bfloat16, kind="ExternalInput")
    w_hbm = nc.dram_tensor("w_ext", [65, cfg.layers * F], mybir.dt.bfloat16,
                           kind="ExternalInput")
    dv_hbm = nc.dram_tensor("dinv_t", [128, NBLK], mybir.dt.float32,
                            kind="ExternalInput")
    i_hbm = nc.dram_tensor("ident", [128, 128], mybir.dt.bfloat16,
                           kind="ExternalInput")
    t_hbm = nc.dram_tensor("iota_t", [128, OVG * OV_CB * 128], mybir.dt.float16,
                           kind="ExternalInput")
    tx_hbm = nc.dram_tensor("iota_x", [128, 2 * CELL * 128], mybir.dt.float16,
                            kind="ExternalInput")
    rc_hbm = nc.dram_tensor("rowcell", [128, SROW], mybir.dt.float16,
                            kind="ExternalInput")
    og_hbm = nc.dram_tensor("ohg", [NBLK * 128, SROW], mybir.dt.float8e4,
                            kind="ExternalInput")
    ohs_hbm = nc.dram_tensor("ohs", [(NBLK // 2) * 80 * 2 * CELL, 128],
                             mybir.dt.float8e4, kind="ExternalInput")
    ovr_hbm = nc.dram_tensor("ovrow", [128, EPADOV // 128], mybir.dt.float16,
                             kind="ExternalInput")
    ovc_hbm = nc.dram_tensor("ovcolr", [128, EPADOV // 16], mybir.dt.int16,
                             kind="ExternalInput")
    out_hbm = nc.dram_tensor("out_pm", [128, NBLK * F], mybir.dt.float32,
                             kind="ExternalOutput")
    zdram = nc.dram_tensor("zdram", [N, 2 * F], mybir.dt.float16)
    cells = nc.dram_tensor("cells", [EPADG, F], mybir.dt.float16)

    with tile.TileContext(nc) as tc:
        with (
            tc.tile_pool(name="const", bufs=1) as cp,
            tc.tile_pool(name="state", bufs=1) as sp,
            tc.tile_pool(name="ohgb", bufs=4) as obp,
            tc.tile_pool(name="oh", bufs=2) as ohp,
            tc.tile_pool(name="mst", bufs=2) as msp,
            tc.tile_pool(name="msg", bufs=2) as mp,
            tc.tile_pool(name="ovm", bufs=2) as ovp,
            tc.tile_pool(name="zb", bufs=2) as zp,
            tc.tile_pool(name="pz", bufs=2, space="PSUM") as pz,
            tc.tile_pool(name="pmg", bufs=2, space="PSUM") as pmg,
            tc.tile_pool(name="psc", bufs=2, space="PSUM") as psc,
            tc.tile_pool(name="pbt", bufs=2, space="PSUM") as pbt,
        ):
            nc.gpsimd.load_library(mlp)

            wt = cp.tile([65, cfg.layers, F], mybir.dt.bfloat16)
            nc.sync.dma_start(wt[:], w_hbm[:].rearrange("p (l f) -> p l f",
                                                        l=cfg.layers))
            dinv = cp.tile([128, NBLK], mybir.dt.float32)
            nc.sync.dma_start(dinv[:], dv_hbm[:])
            ident = cp.tile([128, 128], mybir.dt.bfloat16)
            nc.sync.dma_start(ident[:], i_hbm[:])
            iota = cp.tile([128, OVG * OV_CB * 128], mybir.dt.float16)
            nc.sync.dma_start(iota[:], t_hbm[:])
            iotax = cp.tile([128, 2 * CELL * 128], mybir.dt.float16)
            nc.sync.dma_start(iotax[:], tx_hbm[:])
            rowcell = cp.tile([128, SROW], mybir.dt.float16)
            nc.sync.dma_start(rowcell[:], rc_hbm[:])
            ovrow = cp.tile([128, EPADOV // 128], mybir.dt.float16)
            nc.sync.dma_start(ovrow[:], ovr_hbm[:])
            ovcolr = cp.tile([128, EPADOV // 16], mybir.dt.int16)
            nc.sync.dma_start(ovcolr[:], ovc_hbm[:])

            hA = sp.tile([65, N], mybir.dt.bfloat16, tag="hA")
            hB = sp.tile([65, N], mybir.dt.bfloat16, tag="hB")
            stage = sp.tile([128, NBLK, F], mybir.dt.float32, tag="stage")
            zb16 = sp.tile([128, NBLK, 2 * F], mybir.dt.float16, tag="zb16")

            oh_dt = mybir.dt.float8e4 if OH_FP8 else mybir.dt.float16
            ohsv = ohs_hbm[:].rearrange("(dp s k) o -> dp s k o",
                                        dp=NBLK // 2, s=80)

            def onehot_cell2(dp):
                # scatter one-hot for a PAIR of dest blocks, [s, k2, off]
                # with off innermost so matmul stationary slices [80, 128]
                # stay contiguous (FWL); k2<32: block 2dp, k2>=32: 2dp+1.
                # Either DVE-built from rowcell or streamed from HBM.
                oh = ohp.tile([80, 2 * CELL, 128], oh_dt, tag="oh")
                ship = OH_SHIP == 2 or (OH_SHIP == 1 and dp % 2 == 0)
                if ship:
                    nc.sync.dma_start(oh[:], ohsv[dp])
                    return oh
                ro3 = rowcell[:80, dp * 2 * CELL:(dp + 1) * 2 * CELL] \
                    .rearrange("p k -> p k ()")
                io3 = iotax[:80].rearrange("p (k j) -> p k j", k=2 * CELL)
                a, bb = broadcast_tensor_aps(io3, ro3)
                nc.vector.tensor_tensor(out=oh[:], in0=a, in1=bb,
                                        op=mybir.AluOpType.is_equal)
                return oh

            def onehot_ov4(og):
                # overflow one-hots for a whole og group (OVG dests) in
                # one DVE op: oh[p, dd, c, k]
                oh = ohp.tile([128, OVG, OV_CB, 128], oh_dt, tag="ohov")
                ro3 = ovrow[:, og * OVG * OV_CB:(og + 1) * OVG * OV_CB] \
                    .rearrange("p (dd c) -> p dd c ()", dd=OVG)
                io3 = iota[:].rearrange("p (dd c k) -> p dd c k",
                                        dd=OVG, c=OV_CB)
                a, bb = broadcast_tensor_aps(io3, ro3)
                nc.vector.tensor_tensor(out=oh[:], in0=a, in1=bb,
                                        op=mybir.AluOpType.is_equal)
                return oh

            def one_trip():
                nc.sync.dma_start(hA[:], h_hbm[:])
                nc.vector.memset(hB[64:65, :], 1.0)
                nc.vector.memset(zb16[:, :, F:2 * F], 0.0)

                hcur = hA
                for lay in range(cfg.layers):
                    hnxt = hB if hcur is hA else hA
                    # ---- z~ compute: node-major per-block matmul ----
                    for g8 in range(NBLK // 8):
                        pzt = pz.tile([128, 8, F], mybir.dt.float32, tag="pz")
                        for j in range(8):
                            blk = g8 * 8 + j
                            nc.tensor.matmul(
                                pzt[:, j],
                                hcur[:, blk * 128:(blk + 1) * 128],
                                wt[:, lay], start=True, stop=True)
                        dv3 = dinv[:, g8 * 8:(g8 + 1) * 8] \
                            .rearrange("p c -> p c ()")
                        a, bb = broadcast_tensor_aps(pzt[:], dv3)
                        nc.vector.tensor_tensor(
                            out=zb16[:, g8 * 8:(g8 + 1) * 8, 0:F],
                            in0=a, in1=bb, op=mybir.AluOpType.mult)
                    nc.sync.dma_start(
                        zdram[:].rearrange("(p c) k -> p c k", p=128), zb16[:])

                    # ---- gather phase: src blocks in pairs ----
                    drains = [nc.scalar, nc.scalar, nc.vector]
                    nd = 0
                    for sbp in range(NBLK // 2):
                        ob = obp.tile([128, 2, SROW], mybir.dt.float8e4, tag="ob")
                        nc.sync.dma_start(
                            ob[:], og_hbm[sbp * 256:(sbp + 1) * 256, :]
                            .rearrange("(g p) c -> p g c", g=2))
                        mstg = msp.tile([128, 2, SCH, F], mybir.dt.float16,
                                        tag="mstg")
                        for g in range(2):
                            sb = 2 * sbp + g
                            for c8 in range(0, SCH, 8):
                                w8 = min(8, SCH - c8)
                                pm = pmg.tile([128, 8, F], mybir.dt.float32,
                                              tag="pmg")
                                for j in range(w8):
                                    nc.tensor.matmul(
                                        pm[:, j],
                                        ob[:, g, (c8 + j) * 128:(c8 + j + 1) * 128],
                                        zb16[:, sb, 0:F], start=True, stop=True)
                                eng = drains[nd % 3]
                                nd += 1
                                if eng is nc.scalar:
                                    eng.copy(mstg[:, g, c8:c8 + w8],
                                             pm[:, :w8])
                                else:
                                    eng.tensor_copy(
                                        out=mstg[:, g, c8:c8 + w8],
                                        in_=pm[:, :w8])
                        nc.sync.dma_start(
                            cells[sbp * 2 * SROW:(sbp + 1) * 2 * SROW, :]
                            .rearrange("(g p c) f -> p g c f", g=2, p=128), mstg[:])

                    # ---- scatter phase: per dest block ----
                    cells4 = cells[:].rearrange("(s dp k2) f -> dp s k2 f",
                                                s=NBLK, dp=NBLK // 2)
                    for og in range(NBLK // OVG):
                        novi = OVG * OV_CB * 128
                        ovm = ovp.tile([128, OVG * OV_CB, 2 * F],
                                       mybir.dt.float16, tag="ovm")
                        nc.gpsimd.dma_gather(
                            ovm[:], zdram[:],
                            ovcolr[:, og * (novi // 16):(og + 1) * (novi // 16)],
                            novi, novi, 2 * F, single_packet=False)
                        ohov = onehot_ov4(og)
                        for dd in range(0, OVG, 2):
                            dp = (og * OVG + dd) // 2
                            msgs = mp.tile([80, 2 * CELL, F], mybir.dt.float16,
                                           tag="msgs")
                            nc.sync.dma_start(msgs[:], cells4[dp])
                            oh = onehot_cell2(dp)
                            for half in range(2):
                                d = 2 * dp + half
                                ps = psc.tile([128, F], mybir.dt.float32,
                                              tag="psc")
                                for k in range(CELL):
                                    nc.tensor.matmul(
                                        ps[:], oh[:, half * CELL + k, :],
                                        msgs[:, half * CELL + k],
                                        start=(k == 0), stop=False)
                                for c in range(OV_CB):
                                    nc.tensor.matmul(
                                        ps[:], ohov[:, dd + half, c],
                                        ovm[:, (dd + half) * OV_CB + c, 0:F],
                                        start=False, stop=(c == OV_CB - 1))
                                if lay < cfg.layers - 1:
                                    hm = zp.tile([128, F], mybir.dt.bfloat16,
                                                 tag="hm")
                                    nc.scalar.activation(
                                        hm[:], ps[:],
                                        mybir.ActivationFunctionType.Relu,
                                        scale=dinv[:, d:d + 1])
                                    pbtt = pbt.tile([64, 128], mybir.dt.float32,
                                                    tag="pbt")
                                    nc.tensor.matmul(pbtt[:], hm[:], ident[:],
                                                     start=True, stop=True)
                                    nc.scalar.copy(
                                        hnxt[0:64, d * 128:(d + 1) * 128],
                                        pbtt[:])
                                else:
                                    nc.scalar.activation(
                                        stage[:, d], ps[:],
                                        mybir.ActivationFunctionType.Copy,
                                        scale=dinv[:, d:d + 1])
                    hcur = hnxt
                nc.sync.dma_start(
                    out_hbm[:].rearrange("p (c f) -> p c f", c=NBLK), stage[:])

            for _ in range(trips):
                one_trip()

    nc.compile()
    return nc


def _prep_inputs(cfg: Cfg, x, edge_index, Ws, bs_):
    """Index/layout marshaling + host dinv (index data only)."""
    OV_CB = cfg.OV_CB
    EPADOV = cfg.epadov
    row = np.asarray(edge_index[:, 0], np.int64)
    col = np.asarray(edge_index[:, 1], np.int64)
    d = row >> 7
    s = col >> 7
    cell = s * NBLK + d
    order = np.argsort(cell, kind="stable")
    cs = cell[order]
    counts = np.bincount(cell, minlength=NBLK * NBLK)
    starts = np.cumsum(counts) - counts
    within = np.arange(len(row)) - starts[cs]
    main = within < CELL
    ro, co, wo = row[order], col[order], within
    so, do = s[order], d[order]

    gslot = so * SROW + do * CELL + wo          # src-major grid slot
    gsrc = np.full(EPADG, 999, np.int32)
    gsrc[gslot[main]] = (co & 127)[main]
    rowcell_f = np.full(EPADG, 999.0, np.float32)
    rowcell_f[gslot[main]] = (ro & 127)[main].astype(np.float32)

    # offset-major one-hot per src block, columns permuted so gather-mm
    # chunk j partition p addresses within-sb row v = p*SCH + j
    ohg = (gsrc.reshape(NBLK, SROW)[:, None, :] ==
           np.arange(128, dtype=np.int32)[None, :, None])
    perm = np.arange(SROW).reshape(128, SCH).transpose(1, 0).reshape(-1)
    ohg = np.ascontiguousarray(ohg[:, :, perm]).astype(
        mybir.dt.np(mybir.dt.float8e4))

    # rowcell [128, 2560]: partition = src block s (80 used), free = d*32+k
    rowcell_t = np.full((128, SROW), 999.0, np.float16)
    rowcell_t[:NBLK] = rowcell_f.reshape(NBLK, SROW).astype(np.float16)

    # pre-expanded scatter one-hots [dp, s, k2, off], off innermost (fp8)
    rc3 = rowcell_f.reshape(NBLK, NBLK, CELL)
    eq = rc3[:, :, :, None] == np.arange(128, dtype=np.float32)[None, None, None]
    ohs = eq.transpose(1, 0, 2, 3).reshape(NBLK // 2, 2, NBLK, CELL, 128) \
        .transpose(0, 2, 1, 3, 4).reshape(NBLK // 2, NBLK, 2 * CELL, 128) \
        .astype(mybir.dt.np(mybir.dt.float8e4))

    # overflow edges (dest-bucketed, per-edge gather path)
    ov = ~main
    rov, cov, dov = ro[ov], co[ov], do[ov]
    ocounts = np.bincount(dov, minlength=NBLK)
    assert ocounts.max() <= OV_CB * 128, f"ov overflow: {ocounts.max()}"
    oorder = np.argsort(dov, kind="stable")
    ostarts = np.cumsum(ocounts) - ocounts
    obase = np.repeat(np.arange(NBLK) * OV_CB * 128, ocounts)
    owithin = np.arange(len(rov)) - np.repeat(ostarts, ocounts)
    oslots = obase + owithin
    ovrow = np.full(EPADOV, 999.0, np.float32)
    ovcol = np.zeros(EPADOV, np.int64)
    ovrow[oslots] = (rov & 127)[oorder]
    ovcol[oslots] = cov[oorder]
    ovcolr = ((ovcol & 127) * NBLK + (ovcol >> 7)).astype(np.int16)

    def wrap16(a):
        w = a.reshape(-1, 16).T
        return np.tile(w, (8, 1))

    ovrow_t = np.ascontiguousarray(ovrow.reshape(-1, 128).T.astype(np.float16))
    ovcolr_t = wrap16(ovcolr)

    # dinv from edge counts (index data): deg = bincount of dest node ids
    deg = np.bincount(row, minlength=N).astype(np.float64)
    dinv = np.where(deg > 0, 1.0 / np.sqrt(np.maximum(deg, 1.0)), 0.0)
    dinv_t = np.ascontiguousarray(
        dinv.reshape(NBLK, 128).T.astype(np.float32))   # [128 off, 80 blk]

    # h0 f-major bf16 with ones row folded in for the bias
    h0 = np.zeros((65, N), np.float32)
    h0[:F, :NV] = np.asarray(x, np.float32).T
    h0[F, :] = 1.0
    h0_ext = h0.astype(mybir.dt.np(mybir.dt.bfloat16))

    w_ext = np.zeros((65, len(Ws), F), np.float32)
    for l, (W, b) in enumerate(zip(Ws, bs_)):
        w_ext[:F, l] = np.asarray(W, np.float32).T
        w_ext[F, l] = np.asarray(b, np.float32)
    w_ext = np.ascontiguousarray(w_ext.reshape(65, -1)).astype(
        mybir.dt.np(mybir.dt.bfloat16))

    return {
        "h0_ext": h0_ext,
        "w_ext": w_ext,
        "dinv_t": dinv_t,
        "ident": np.eye(128, dtype=mybir.dt.np(mybir.dt.bfloat16)),
        "iota_t": np.tile(np.tile(np.arange(128, dtype=np.float16),
                                  cfg.OVG * OV_CB), (128, 1)),
        "iota_x": np.tile(np.arange(128, dtype=np.float16), (128, 2 * CELL)),
        "rowcell": rowcell_t,
        "ohg": np.ascontiguousarray(ohg.reshape(NBLK * 128, SROW)),
        "ohs": np.ascontiguousarray(
            ohs.reshape((NBLK // 2) * 80 * 2 * CELL, 128)),
        "ovrow": ovrow_t,
        "ovcolr": ovcolr_t,
    }


def _unpack_output(out_pm):
    o = out_pm.reshape(128, NBLK, F).transpose(1, 0, 2).reshape(N, F)
    return o[:NV]


def kernel(x, edge_index, W1, b1, W2, b2, W3, b3):
    x = np.asarray(x)
    edge_index = np.asarray(edge_index)
    Ws = [np.asarray(W1), np.asarray(W2), np.asarray(W3)]
    bs_ = [np.asarray(b1), np.asarray(b2), np.asarray(b3)]
    nb = x.shape[0]
    assert x.shape == (B, NV, F) and edge_index.shape == (B, E, 2)

    cfg = make_cfg(edge_index)
    in_maps = [_prep_inputs(cfg, x[g], edge_index[g], Ws, bs_)
               for g in range(nb)]
    nc = _build(cfg)
    try:
        res = run_bass_kernel_spmd(nc, in_maps, CORES).results
    except Exception:
        # transient NRT device wedge recovers on a fresh attempt
        res = run_bass_kernel_spmd(nc, in_maps, CORES).results
    out = np.stack([_unpack_output(res[g]["out_pm"]) for g in range(nb)])
    return out.astype(np.float32)
